# revision 1
# baseline (speedup 1.0000x reference)
"""CrossAttention Trainium2 kernel.

Reference computation (per batch b):
  q = x @ Wq; k = ctx @ Wk; v = ctx @ Wv   (multi-head, H=8, DH=64)
  out = softmax(q k^T / sqrt(DH)) v @ Wo + bo, rows >= seq_len zeroed.

Strategy: only rows < seq_len are computed ("ragged"); valid query tiles
(512 rows) are distributed across the 8 cores with a statically uniform
per-core structure: every core runs CAP query tiles, the first CAP_A of
which read KV slot A and the rest KV slot B. Which batch each slot holds
is per-core DATA (context tensors), so one SPMD program serves all cores.

On-chip layout is fully "transposed" (sequence on the free dim) so no
on-chip transposes are needed:
  xT [DQ, Lt] -> qT = Wq^T xT [INNER, Lt]
  kT = Wk^T ctxT [INNER, S];  v_aug = (ctx_aug @ Wv_aug) [S, 8*(DH+1)]
  scoresT_h [S, Lt] = kT_h^T qT_h  (per head, K=DH)
  expT = exp(scoresT)  (no max subtraction needed: logits ~ N(0,1))
  avT_h [DH+1, Lt] = v_aug_h^T expT_h  (extra ones column -> row DH = colsum)
  normalize per head via reciprocal + partition-broadcast + multiply
  out [Lt, DQ] = outT_aug^T @ Wo_aug (ones K-row adds bo)
Matmuls run in float32r (full-rate fp32, ~1e-4 rel err).
"""

import math
import sys

sys.path.insert(0, "/opt/trn_rl_repo")

import numpy as np

B, L, S = 8, 8192, 512
DQ, DC = 256, 768
H, DH = 8, 64
INNER = H * DH
TL = 512          # query rows per tile
N_CORES = 8
DCA = DC + 1      # ctx augmented with a ones row
WVN = H * (DH + 1)  # 520: v augmented with a ones column per head


def _plan(nt):
    """Choose (CAP_A, CAP_B) and per-core pieces. Returns
    (cap_a, cap_b, cores) where cores is a list of 8 entries
    [(batch_a, tile0_a), (batch_b, tile0_b)] (batch -1 = padding)."""
    best = None
    lo = max(1, math.ceil(sum(nt) / N_CORES))
    for cap in range(lo, max(max(nt), lo) + 9):
        # NB=1: whole batches in CAP-size pieces
        if sum(math.ceil(n / cap) for n in nt) <= N_CORES:
            cost = cap * 18 + 12
            if best is None or cost < best[0]:
                best = (cost, cap, 0, None)
        # NB=2 split
        for a in range(cap - 1, 0, -1):
            b = cap - a
            opts = []
            for n in nt:
                o = []
                for ka in range(0, N_CORES + 1):
                    rem = n - a * ka
                    kb = max(0, math.ceil(rem / b))
                    if kb <= N_CORES:
                        o.append((ka, kb))
                opts.append(o)

            found = None

            def dfs(i, ta, tb, acc):
                nonlocal found
                if found is not None:
                    return
                if i == len(opts):
                    found = list(acc)
                    return
                for ka, kb in opts[i]:
                    if ta + ka <= N_CORES and tb + kb <= N_CORES:
                        acc.append((ka, kb))
                        dfs(i + 1, ta + ka, tb + kb, acc)
                        acc.pop()
                        if found is not None:
                            return

            dfs(0, 0, 0, [])
            if found is not None:
                cost = cap * 18 + 24
                if best is None or cost < best[0]:
                    best = (cost, cap, 1, (a, b, found))
                break  # larger a preferred; next a adds nothing
    assert best is not None
    _, cap, kind, info = best
    if kind == 0:
        # NB=1: emit as (a=cap, b=0-like) with slot B duplicating slot A
        pieces_a = []
        for bi, n in enumerate(nt):
            for j in range(math.ceil(n / cap)):
                pieces_a.append((bi, j * cap))
        while len(pieces_a) < N_CORES:
            pieces_a.append((-1, 0))
        cores = [[pa, (-1, 0)] for pa in pieces_a]
        return cap, 0, cores
    a, bsz, ks = info
    pieces_a, pieces_b = [], []
    for bi, n in enumerate(nt):
        ka, kb = ks[bi]
        t = 0
        for _ in range(ka):
            pieces_a.append((bi, t))
            t += a
        for _ in range(kb):
            pieces_b.append((bi, t))
            t += bsz
    while len(pieces_a) < N_CORES:
        pieces_a.append((-1, 0))
    while len(pieces_b) < N_CORES:
        pieces_b.append((-1, 0))
    cores = [[pieces_a[i], pieces_b[i]] for i in range(N_CORES)]
    return a, bsz, cores


_PROG_CACHE = {}


def _build_program(cap_a, cap_b):
    import concourse.mybir as mybir
    import concourse.tile as tile
    from concourse import bacc

    f32 = mybir.dt.float32
    f32r = mybir.dt.float32r
    CAP = cap_a + cap_b
    NSLOT = 2 if cap_b > 0 else 1

    nc = bacc.Bacc("TRN2", target_bir_lowering=False, debug=False,
                   num_devices=N_CORES)
    xT = nc.declare_dram_parameter("xT", [DQ, CAP * TL], f32r, isOutput=False)
    ctxs = [nc.declare_dram_parameter(f"ctx{s}", [DC, S], f32r, isOutput=False)
            for s in range(NSLOT)]
    wq = nc.declare_dram_parameter("wq", [DQ, INNER], f32r, isOutput=False)
    wk = nc.declare_dram_parameter("wk", [DC, INNER], f32r, isOutput=False)
    wv = nc.declare_dram_parameter("wv", [DC, INNER], f32r, isOutput=False)
    wo = nc.declare_dram_parameter("wo", [INNER, DQ], f32r, isOutput=False)
    bob = nc.declare_dram_parameter("bob", [128, DQ], f32, isOutput=False)
    vones = nc.declare_dram_parameter("vones", [128, 8, 1], f32r, isOutput=False)
    y = nc.declare_dram_parameter("y", [CAP * TL, DQ], f32, isOutput=True)

    with tile.TileContext(nc) as tc:
        with (
            tc.tile_pool(name="wpool", bufs=1) as wpool,
            tc.tile_pool(name="kvpool", bufs=1) as kvpool,
            tc.tile_pool(name="ctxpool", bufs=1) as ctxpool,
            tc.tile_pool(name="mpool", bufs=4) as mpool,
            tc.tile_pool(name="qpool", bufs=3) as qpool,
            tc.tile_pool(name="epool", bufs=5) as epool,
            tc.tile_pool(name="opool", bufs=2) as opool,
            tc.tile_pool(name="spool", bufs=4) as spool,
            tc.tile_pool(name="ypool", bufs=4) as ypool,
            tc.tile_pool(name="ps_big", bufs=2, space="PSUM") as ps_big,
            tc.tile_pool(name="ps_sc", bufs=2, space="PSUM") as ps_sc,
            tc.tile_pool(name="ps_av", bufs=2, space="PSUM") as ps_av,
        ):
            # ---- load weights; DMA order tracks first-use order:
            # wk+ctx0 (KV slot A) -> wq+x0/x1 (tile-0 qT) -> rest
            wk_sb = [wpool.tile([128, INNER], f32r, tag=f"wk{i}", name=f"wk{i}") for i in range(6)]
            for i in range(6):
                nc.sync.dma_start(wk_sb[i][:], wk[i * 128:(i + 1) * 128, :])
            ctx_tiles = {}
            for s in range(NSLOT):
                ctx_tiles[s] = [ctxpool.tile([128, S], f32r, tag=f"ctx{s}_{i}", name=f"ctx{s}_{i}")
                                for i in range(6)]
            for i in range(6):
                nc.sync.dma_start(ctx_tiles[0][i][:], ctxs[0][i * 128:(i + 1) * 128, :])
            wq_sb = [wpool.tile([128, INNER], f32r, tag=f"wq{i}", name=f"wq{i}") for i in range(2)]
            for i in range(2):
                nc.sync.dma_start(wq_sb[i][:], wq[i * 128:(i + 1) * 128, :])
            pre_x = {}
            for t in range(min(2, CAP)):
                xt_t = [mpool.tile([128, TL], f32r, tag=f"x{kc}", name=f"x{kc}")
                        for kc in range(2)]
                for kc in range(2):
                    nc.sync.dma_start(
                        xt_t[kc][:], xT[kc * 128:(kc + 1) * 128, t * TL:(t + 1) * TL])
                pre_x[t] = xt_t
            wv_sb = [wpool.tile([128, INNER], f32r, tag=f"wv{i}", name=f"wv{i}") for i in range(6)]
            for i in range(6):
                nc.sync.dma_start(wv_sb[i][:], wv[i * 128:(i + 1) * 128, :])
            for s in range(1, NSLOT):
                for i in range(6):
                    nc.sync.dma_start(ctx_tiles[s][i][:], ctxs[s][i * 128:(i + 1) * 128, :])
            wo_sb = [wpool.tile([128, DQ], f32r, tag=f"wo{i}", name=f"wo{i}") for i in range(4)]
            for i in range(4):
                nc.sync.dma_start(wo_sb[i][:], wo[i * 128:(i + 1) * 128, :])
            bob_sb = wpool.tile([128, DQ], f32, tag="bob", name="bob")
            nc.sync.dma_start(bob_sb[:], bob[:])

            # ---- KV phase per slot (emitted lazily before its tile group,
            # so slot-B KV doesn't block slot-A tiles on the in-order PE) ----
            kT = {}
            vA = {}
            def kv_phase(s):
                    ctx_sb = ctx_tiles[s]

                    kT[s] = [kvpool.tile([128, S], f32r, tag=f"kT{s}_{m}", name=f"kT{s}_{m}")
                             for m in range(4)]
                    for m in range(4):
                        pk = ps_big.tile([128, S], f32, tag="big", name="big")
                        for kc in range(6):
                            nc.tensor.matmul(
                                pk[:], wk_sb[kc][:, m * 128:(m + 1) * 128],
                                ctx_sb[kc][:], start=(kc == 0), stop=(kc == 5))
                        nc.vector.tensor_copy(kT[s][m][:], pk[:])
                    vA[s] = [kvpool.tile([128, WVN], f32r, tag=f"v{s}_{sc}", name=f"v{s}_{sc}")
                             for sc in range(4)]
                    for sc in range(4):
                        pv = ps_big.tile([128, 512], f32, tag="big", name="big")
                        for kc in range(6):
                            nc.tensor.matmul(
                                pv[:],
                                ctx_sb[kc][:, sc * 128:(sc + 1) * 128],
                                wv_sb[kc][:], start=(kc == 0), stop=(kc == 5))
                        vdst = vA[s][sc][:].rearrange("p (h d) -> p h d", d=DH + 1)
                        nc.vector.tensor_copy(
                            vdst[:, :, 0:DH],
                            pv[:].rearrange("p (h d) -> p h d", d=DH))
                        nc.sync.dma_start(vdst[:, :, DH:DH + 1], vones[:])

            # ---- main loop over query tiles ----
            pending_oproj = []

            def emit_oproj(tt, outT_t):
                for lsub in range(4):
                    po = ps_av.tile([128, DQ], f32, tag="av", name="av")
                    for kc in range(4):
                        nc.tensor.matmul(
                            po[:], outT_t[kc][:, lsub * 128:(lsub + 1) * 128],
                            wo_sb[kc][:], start=(kc == 0), stop=(kc == 3))
                    yt = ypool.tile([128, DQ], f32, tag="y", name="y")
                    nc.vector.tensor_add(yt[:], po[:], bob_sb[:])
                    nc.sync.dma_start(
                        y[tt * TL + lsub * 128: tt * TL + (lsub + 1) * 128, :],
                        yt[:])

            for t in range(CAP):
                s = 0 if t < cap_a else 1
                if t == 0:
                    kv_phase(0)
                if t == cap_a and NSLOT > 1:
                    kv_phase(1)
                elif t == 0 and NSLOT > 1 and cap_a == 0:
                    kv_phase(1)
                if t in pre_x:
                    xt = pre_x.pop(t)
                else:
                    xt = [mpool.tile([128, TL], f32r, tag=f"x{kc}", name=f"x{kc}") for kc in range(2)]
                    for kc in range(2):
                        nc.sync.dma_start(
                            xt[kc][:], xT[kc * 128:(kc + 1) * 128, t * TL:(t + 1) * TL])
                qT = [qpool.tile([128, TL], f32r, tag=f"q{m}", name=f"q{m}") for m in range(4)]
                for m in range(4):
                    pq = ps_big.tile([128, TL], f32, tag="big", name="big")
                    for kc in range(2):
                        nc.tensor.matmul(
                            pq[:], wq_sb[kc][:, m * 128:(m + 1) * 128],
                            xt[kc][:], start=(kc == 0), stop=(kc == 1))
                    nc.vector.tensor_copy(qT[m][:], pq[:])

                outT = [opool.tile([128, TL], f32r, tag=f"o{m}", name=f"o{m}") for m in range(4)]

                def emit_scores(h):
                    c, half = h // 2, h % 2
                    expT = []
                    for g in range(2):
                        psc = ps_sc.tile([128, 2, TL], f32, tag="sc", name="sc")
                        for j in range(2):
                            sc = g * 2 + j
                            nc.tensor.matmul(
                                psc[:, j, :],
                                kT[s][c][half * 64:(half + 1) * 64,
                                         sc * 128:(sc + 1) * 128],
                                qT[c][half * 64:(half + 1) * 64, :],
                                start=True, stop=True)
                        e = epool.tile([128, 2, TL], f32r, tag=f"e{g}", name=f"e{g}")
                        nc.scalar.activation(
                            e[:], psc[:], mybir.ActivationFunctionType.Exp)
                        expT.extend([e[:, 0, :], e[:, 1, :]])
                    return expT

                # 1-head lookahead: emit next head's scores before this head's
                # AV so the in-order PE never stalls ACT
                pend = emit_scores(0)
                while pending_oproj:
                    emit_oproj(*pending_oproj.pop(0))
                for h in range(H):
                    c, half = h // 2, h % 2
                    expT = pend
                    if h + 1 < H:
                        pend = emit_scores(h + 1)
                    pav = ps_av.tile([DH + 1, TL], f32, tag="av", name="av")
                    for sc in range(4):
                        nc.tensor.matmul(
                            pav[:], vA[s][sc][:, h * (DH + 1):(h + 1) * (DH + 1)],
                            expT[sc], start=(sc == 0), stop=(sc == 3))
                    rp = spool.tile([1, TL], f32, tag="rp", name="rp")
                    nc.vector.reciprocal(rp[:], pav[DH:DH + 1, :])
                    bc = spool.tile([64, TL], f32, tag="bc", name="bc")
                    nc.gpsimd.partition_broadcast(bc[:], rp[0:1, :])
                    nc.vector.tensor_mul(
                        outT[c][half * 64:(half + 1) * 64, :],
                        pav[0:DH, :], bc[:])

                pending_oproj.append((t, outT))
            while pending_oproj:
                emit_oproj(*pending_oproj.pop(0))
    nc.compile()
    return nc


def kernel(x, context, seq_lens, Wq, Wk, Wv, Wo, bo):
    from concourse.bass_utils import run_bass_kernel_spmd

    x = np.asarray(x, dtype=np.float32)
    context = np.asarray(context, dtype=np.float32)
    seq_lens = np.asarray(seq_lens, dtype=np.int32)
    Wq = np.asarray(Wq, dtype=np.float32)
    Wk = np.asarray(Wk, dtype=np.float32)
    Wv = np.asarray(Wv, dtype=np.float32)
    Wo = np.asarray(Wo, dtype=np.float32)
    bo = np.asarray(bo, dtype=np.float32)

    lens = np.clip(seq_lens, 1, L)
    nt = [int(math.ceil(int(n) / TL)) for n in lens]
    cap_a, cap_b, cores = _plan(nt)
    CAP = cap_a + cap_b
    NSLOT = 2 if cap_b > 0 else 1

    key = (cap_a, cap_b)
    if key not in _PROG_CACHE:
        _PROG_CACHE[key] = _build_program(cap_a, cap_b)
    nc = _PROG_CACHE[key]

    # shared (replicated) weights
    scale = 1.0 / math.sqrt(DH)
    wq_in = (Wq * scale).astype(np.float32)
    wv_in = np.ascontiguousarray(Wv.astype(np.float32))
    wo_in = np.ascontiguousarray(Wo.astype(np.float32))
    bob_in = np.broadcast_to(bo[None, :], (128, DQ)).copy()
    vones_in = np.ones((128, 8, 1), dtype=np.float32)

    in_maps = []
    for core in range(N_CORES):
        xt_core = np.zeros((CAP * TL, DQ), dtype=np.float32)
        m = {}
        for sidx in range(NSLOT):
            bi, t0 = cores[core][sidx]
            npieces = cap_a if sidx == 0 else cap_b
            if bi >= 0:
                r0 = t0 * TL
                r1 = min(r0 + npieces * TL, L)
                if r1 > r0:
                    off = sidx * cap_a * TL
                    xt_core[off:off + (r1 - r0)] = x[bi, r0:r1]
                cb = context[bi]
            else:
                cb = context[0]
            m[f"ctx{sidx}"] = np.ascontiguousarray(cb.T)
        m["xT"] = np.ascontiguousarray(xt_core.T)
        m["wq"] = wq_in
        m["wk"] = Wk
        m["wv"] = wv_in
        m["wo"] = wo_in
        m["bob"] = bob_in
        m["vones"] = vones_in
        in_maps.append(m)

    res = run_bass_kernel_spmd(nc, in_maps, list(range(N_CORES)))

    out = np.zeros((B, L, DQ), dtype=np.float32)
    for core in range(N_CORES):
        yc = res.results[core]["y"]
        for sidx in range(NSLOT):
            bi, t0 = cores[core][sidx]
            if bi < 0:
                continue
            npieces = cap_a if sidx == 0 else cap_b
            r0 = t0 * TL
            r1 = min(r0 + npieces * TL, int(lens[bi]))
            if r1 > r0:
                off = sidx * cap_a * TL
                out[bi, r0:r1] = yc[off:off + (r1 - r0)]
    return out



# revision 5
# speedup vs baseline: 1.0783x; 1.0783x over previous
"""CrossAttention Trainium2 kernel (v3: bf16 pipeline, l-major AV).

Reference (per batch b): q = x@Wq; k = ctx@Wk; v = ctx@Wv (H=8, DH=64)
  out = softmax(q k^T / sqrt(DH)) v @ Wo + bo, rows >= seq_len zeroed.

Valid 512-row query tiles are packed across 8 cores with a per-core slot
structure: each core runs CAP tiles; slot j holds sizes[j] consecutive
tiles reading KV buffer j (which batch a (core, slot) holds is data, so
one SPMD program serves all cores). A planner picks slot sizes (up to 3
slots) minimizing CAP then slots; for the staged seq_lens it packs 56
tiles as 8 cores x 7 tiles with zero slot waste.

Engine plan (per tile of 512 queries):
- PE (bf16, 1 cyc/row): qT 4096 + scores 16384 + AV 8320 + transpose
  2048 + oproj 4096 cycles; KV 24576/slot.
- ACT: exp psum->bf16, 16 instrs of [128,2,512] (~16.6us) - bottleneck;
  scores for tile t+1 are emitted before the AV tail of tile t so the
  ACT queue never starves.
- AV is l-major (out [l, h, 65], v augmented with a ones column) so the
  softmax denominator lands as a per-partition scalar: reciprocal
  [128,4,1] + broadcast_to multiply on DVE; then PE-transpose (identity
  matmul) to outT for the bf16 output projection (+ K=1 bias row).
fp8 was measured (numpy mirror of this exact dataflow) at 4e-2..8e-2
rel err - above the 2e-2 gate - so everything stays bf16 (4.9e-3).
"""

import math
import sys

sys.path.insert(0, "/opt/trn_rl_repo")

import numpy as np
import ml_dtypes

B, L, S = 8, 8192, 512
DQ, DC = 256, 768
H, DH = 8, 64
INNER = H * DH
TL = 512
N_CORES = 8

BF16 = ml_dtypes.bfloat16
EXP_SCALE = 1.0 / 8.0          # 1/sqrt(DH)


def _plan(nt):
    """Pick slot sizes (<=3 slots) and per-core pieces.

    Returns (sizes, cores): sizes = tuple of slot lengths (tiles); cores =
    list of 8 entries, each a list of len(sizes) pieces (batch, tile0),
    batch -1 = padding."""
    T = sum(nt)
    lo = max(1, math.ceil(T / N_CORES))
    order = sorted(range(len(nt)), key=lambda i: -nt[i])

    def combos(n, sizes):
        k = len(sizes)
        out = []
        maxx = [min(8, math.ceil(n / s) + 1) for s in sizes]

        def rec(j, x, tot):
            if j == k:
                if tot >= n and all(
                    x[i] == 0 or tot - sizes[i] < n for i in range(k)
                ):
                    out.append(tuple(x))
                return
            for c in range(0, maxx[j] + 1):
                x.append(c)
                rec(j + 1, x, tot + c * sizes[j])
                x.pop()

        rec(0, [], 0)
        return out

    def feasible(sizes):
        k = len(sizes)
        budget = [N_CORES] * k
        pick = [None] * len(nt)

        def rec(bi):
            if bi == len(nt):
                return True
            i = order[bi]
            for x in combos(nt[i], sizes):
                if all(budget[j] >= x[j] for j in range(k)):
                    for j in range(k):
                        budget[j] -= x[j]
                    pick[i] = x
                    if rec(bi + 1):
                        return True
                    for j in range(k):
                        budget[j] += x[j]
            return False

        return pick if rec(0) else None

    best = None
    for cap in range(lo, lo + 10):
        for k in (1, 2, 3):
            parts = set()
            if k == 1:
                parts.add((cap,))
            elif k == 2:
                for a in range(cap - 1, 0, -1):
                    if a >= cap - a:
                        parts.add((a, cap - a))
            else:
                for a in range(cap - 2, 0, -1):
                    for b in range(min(a, cap - a - 1), 0, -1):
                        c = cap - a - b
                        if 0 < c <= b:
                            parts.add((a, b, c))
            for sizes in sorted(parts, reverse=True):
                pick = feasible(sizes)
                if pick is not None:
                    best = (sizes, pick)
                    break
            if best:
                break
        if best:
            break
    assert best is not None
    sizes, pick = best
    k = len(sizes)
    slot_pieces = [[] for _ in range(k)]
    for i, n in enumerate(nt):
        off = 0
        for j in range(k):
            for _ in range(pick[i][j]):
                slot_pieces[j].append((i, off))
                off += sizes[j]
    for j in range(k):
        while len(slot_pieces[j]) < N_CORES:
            slot_pieces[j].append((-1, 0))
    cores = [[slot_pieces[j][c] for j in range(k)] for c in range(N_CORES)]
    return sizes, cores


_PROG_CACHE = {}


def _build_program(sizes, has_bias):
    import concourse.mybir as mybir
    import concourse.tile as tile
    from concourse import bacc

    f32 = mybir.dt.float32
    bf16 = mybir.dt.bfloat16
    Exp = mybir.ActivationFunctionType.Exp
    NSLOT = len(sizes)
    CAP = sum(sizes)
    slot_of = []
    for j, s in enumerate(sizes):
        slot_of += [j] * s

    nc = bacc.Bacc("TRN2", target_bir_lowering=False, debug=False,
                   num_devices=N_CORES)
    x16 = nc.declare_dram_parameter("x16", [128, 2, CAP, TL], bf16,
                                    isOutput=False)
    ctxs = [nc.declare_dram_parameter(f"ctx{j}", [128, 6, S], bf16,
                                      isOutput=False) for j in range(NSLOT)]
    wq16 = nc.declare_dram_parameter("wq16", [128, 2, INNER], bf16,
                                     isOutput=False)
    wk16 = nc.declare_dram_parameter("wk16", [128, 6, INNER], bf16,
                                     isOutput=False)
    wv16 = nc.declare_dram_parameter("wv16", [128, 6, INNER], bf16,
                                     isOutput=False)
    wo16 = nc.declare_dram_parameter("wo16", [128, 4, DQ], bf16,
                                     isOutput=False)
    if has_bias:
        bo16 = nc.declare_dram_parameter("bo16", [1, DQ], bf16,
                                         isOutput=False)
    id16 = nc.declare_dram_parameter("id16", [128, 128], bf16,
                                     isOutput=False)
    y = nc.declare_dram_parameter("y", [CAP * TL, DQ], f32, isOutput=True)

    with tile.TileContext(nc) as tc:
        with (
            tc.tile_pool(name="wpool", bufs=1) as wpool,
            tc.tile_pool(name="kvpool", bufs=1) as kvpool,
            tc.tile_pool(name="mpool", bufs=3) as mpool,
            tc.tile_pool(name="qpool", bufs=2) as qpool,
            tc.tile_pool(name="epool", bufs=2) as epool,
            tc.tile_pool(name="apool", bufs=2) as apool,
            tc.tile_pool(name="opool", bufs=2) as opool,
            tc.tile_pool(name="spool", bufs=2) as spool,
            tc.tile_pool(name="ypool", bufs=3) as ypool,
            tc.tile_pool(name="ps_sc", bufs=2, space="PSUM") as ps_sc,
            tc.tile_pool(name="ps_av", bufs=1, space="PSUM") as ps_av,
            tc.tile_pool(name="ps_q", bufs=1, space="PSUM") as ps_q,
            tc.tile_pool(name="ps_tr", bufs=1, space="PSUM") as ps_tr,
        ):
            # ---- weights / constants (DMA order ~ first-use order) ----
            wk_sb = wpool.tile([128, 6, INNER], bf16, tag="wk", name="wk")
            nc.sync.dma_start(wk_sb[:], wk16[:])
            ctx_sb = [kvpool.tile([128, 6, S], bf16, tag=f"ctx{j}",
                                  name=f"ctx{j}") for j in range(NSLOT)]
            nc.sync.dma_start(ctx_sb[0][:], ctxs[0][:])
            wq_sb = wpool.tile([128, 2, INNER], bf16, tag="wq", name="wq")
            nc.sync.dma_start(wq_sb[:], wq16[:])
            wv_sb = wpool.tile([128, 6, INNER], bf16, tag="wv", name="wv")
            nc.sync.dma_start(wv_sb[:], wv16[:])
            pre_x = {}
            for t in range(min(2, CAP)):
                xt = mpool.tile([128, 2, TL], bf16, tag="x", name="x")
                nc.sync.dma_start(xt[:], x16[:, :, t, :])
                pre_x[t] = xt
            for j in range(1, NSLOT):
                nc.sync.dma_start(ctx_sb[j][:], ctxs[j][:])
            wo_sb = wpool.tile([128, 4, DQ], bf16, tag="wo", name="wo")
            nc.sync.dma_start(wo_sb[:], wo16[:])
            if has_bias:
                bo_sb = wpool.tile([1, DQ], bf16, tag="bo", name="bo")
                nc.sync.dma_start(bo_sb[:], bo16[:])
                ones1 = wpool.tile([1, 128], bf16, tag="on", name="on")
                nc.gpsimd.memset(ones1[:], 1.0)
            id_sb = wpool.tile([128, 128], bf16, tag="id", name="id")
            nc.sync.dma_start(id_sb[:], id16[:])

            # ---- KV phase per slot (emitted lazily before first tile) ----
            kT = {}
            vT = {}

            def kv_phase(j):
                # kT[j]: [128(par*64+d), 4(hpair), S] bf16
                kT[j] = kvpool.tile([128, 4, S], bf16, tag=f"kT{j}",
                                    name=f"kT{j}")
                for m in range(4):
                    pk = ps_q.tile([128, TL], f32, tag="pq", name="pq")
                    for kc in range(6):
                        nc.tensor.matmul(
                            pk[:], wk_sb[:, kc, m * 128:(m + 1) * 128],
                            ctx_sb[j][:, kc, :], start=(kc == 0),
                            stop=(kc == 5))
                    nc.vector.tensor_copy(kT[j][:, m, :], pk[:])
                # vT[j]: [128(s), 4(schunk), 8(h), 65] bf16, col 64 = 1.0
                vT[j] = kvpool.tile([128, 4, 8, DH + 1], bf16, tag=f"v{j}",
                                    name=f"v{j}")
                nc.gpsimd.memset(vT[j][:, :, :, DH:DH + 1], 1.0)
                for sc in range(4):
                    pv = ps_q.tile([128, TL], f32, tag="pq", name="pq")
                    for kc in range(6):
                        nc.tensor.matmul(
                            pv[:],
                            ctx_sb[j][:, kc, sc * 128:(sc + 1) * 128],
                            wv_sb[:, kc, :], start=(kc == 0), stop=(kc == 5))
                    nc.vector.tensor_copy(
                        vT[j][:, sc, :, 0:DH],
                        pv[:].rearrange("p (h d) -> p h d", d=DH))

            # ---- per-tile pieces ----
            def emit_qt(t, xt):
                qT = [qpool.tile([128, TL], bf16, tag=f"q{m}", name=f"q{m}")
                      for m in range(4)]
                for m in range(4):
                    pq = ps_q.tile([128, TL], f32, tag="pq", name="pq")
                    for kc in range(2):
                        nc.tensor.matmul(
                            pq[:], wq_sb[:, kc, m * 128:(m + 1) * 128],
                            xt[:, kc, :], start=(kc == 0), stop=(kc == 1))
                    nc.vector.tensor_copy(qT[m][:], pq[:])
                return qT

            def emit_scores(t, j, qT):
                # returns e tiles: e[h][g] bf16 [128, 2, TL]
                e = []
                for h in range(H):
                    c, par = h // 2, h % 2
                    rhs = qT[c][64 * par:64 * par + 64, :]
                    eh = []
                    for g in range(2):
                        psc = ps_sc.tile([128, 2, TL], f32, tag="sc",
                                         name="sc")
                        for i in range(2):
                            sc = 2 * g + i
                            nc.tensor.matmul(
                                psc[:, i, :],
                                kT[j][64 * par:64 * par + 64, c,
                                      sc * 128:(sc + 1) * 128],
                                rhs, start=True, stop=True)
                        e16 = epool.tile([128, 2, TL], bf16, tag=f"e{h}{g}",
                                         name=f"e{h}{g}")
                        nc.scalar.activation(e16[:], psc[:], Exp,
                                             scale=EXP_SCALE)
                        eh.append(e16)
                    e.append(eh)
                return e

            def emit_tail(t, j, e):
                # AV + normalize + transpose + oproj + y for tile t
                outT = opool.tile([128, 4, 4, 128], bf16, tag="oT",
                                  name="oT")
                for lc in range(4):
                    pav = [ps_av.tile([128, 4, DH + 1], f32, tag=f"va{a}",
                                      name=f"va{a}") for a in range(2)]
                    for h in range(H):
                        a, hh = h // 4, h % 4
                        for sc in range(4):
                            nc.tensor.matmul(
                                pav[a][:, hh, :],
                                e[h][sc // 2][:, sc % 2,
                                              lc * 128:(lc + 1) * 128],
                                vT[j][:, sc, h, :],
                                start=(sc == 0), stop=(sc == 3))
                    rp = spool.tile([128, 8, 1], f32, tag="rp", name="rp")
                    av = apool.tile([128, 8, DH], bf16, tag=f"av{lc}",
                                    name=f"av{lc}")
                    for a in range(2):
                        nc.vector.reciprocal(rp[:, 4 * a:4 * a + 4, :],
                                             pav[a][:, :, DH:DH + 1])
                        nc.vector.tensor_tensor(
                            av[:, 4 * a:4 * a + 4, :], pav[a][:, :, 0:DH],
                            rp[:, 4 * a:4 * a + 4, :].broadcast_to(
                                [128, 4, DH]),
                            mybir.AluOpType.mult)
                    ptr = ps_tr.tile([128, 4, 128], bf16, tag="tr",
                                     name="tr")
                    for ic in range(4):
                        nc.tensor.transpose(
                            ptr[:, ic, :], av[:, 2 * ic:2 * ic + 2, :],
                            id_sb[:])
                    nc.vector.tensor_copy(outT[:, :, lc, :], ptr[:])
                for ls in range(4):
                    po = ps_q.tile([128, TL], f32, tag="pq", name="pq")
                    for kc in range(4):
                        nc.tensor.matmul(po[:, 0:DQ], outT[:, kc, ls, :],
                                         wo_sb[:, kc, :], start=(kc == 0),
                                         stop=(kc == 3 and not has_bias))
                    if has_bias:
                        nc.tensor.matmul(po[:, 0:DQ], ones1[:], bo_sb[:],
                                         start=False, stop=True)
                    yt = ypool.tile([128, DQ], f32, tag="y", name="y")
                    nc.vector.tensor_copy(yt[:], po[:, 0:DQ])
                    nc.sync.dma_start(
                        y[t * TL + ls * 128:t * TL + (ls + 1) * 128, :],
                        yt[:])

            # ---- main loop: scores run one tile ahead of the tail ----
            pend = None       # (t, j, e) awaiting tail
            for t in range(CAP):
                j = slot_of[t]
                if t == 0 or slot_of[t - 1] != j:
                    kv_phase(j)
                xt = pre_x.pop(t, None)
                if xt is None:
                    xt = mpool.tile([128, 2, TL], bf16, tag="x", name="x")
                    nc.sync.dma_start(xt[:], x16[:, :, t, :])
                if t + 2 < CAP and (t + 2) not in pre_x:
                    xt2 = mpool.tile([128, 2, TL], bf16, tag="x", name="x")
                    nc.sync.dma_start(xt2[:], x16[:, :, t + 2, :])
                    pre_x[t + 2] = xt2
                qT = emit_qt(t, xt)
                e = emit_scores(t, j, qT)
                if pend is not None:
                    emit_tail(*pend)
                pend = (t, j, e)
            emit_tail(*pend)
    nc.compile()
    return nc


def kernel(x, context, seq_lens, Wq, Wk, Wv, Wo, bo):
    from concourse.bass_utils import run_bass_kernel_spmd

    x = np.asarray(x, dtype=np.float32)
    context = np.asarray(context, dtype=np.float32)
    seq_lens = np.asarray(seq_lens, dtype=np.int32)
    Wq = np.asarray(Wq, dtype=np.float32)
    Wk = np.asarray(Wk, dtype=np.float32)
    Wv = np.asarray(Wv, dtype=np.float32)
    Wo = np.asarray(Wo, dtype=np.float32)
    bo = np.asarray(bo, dtype=np.float32)

    lens = np.clip(seq_lens, 1, L)
    nt = [int(math.ceil(int(n) / TL)) for n in lens]
    sizes, cores = _plan(nt)
    NSLOT = len(sizes)
    CAP = sum(sizes)
    has_bias = bool(np.any(bo != 0.0))

    key = (sizes, has_bias)
    if key not in _PROG_CACHE:
        _PROG_CACHE[key] = _build_program(sizes, has_bias)
    nc = _PROG_CACHE[key]

    wq_in = np.ascontiguousarray(
        Wq.reshape(2, 128, INNER).transpose(1, 0, 2)).astype(BF16)
    wk_in = np.ascontiguousarray(
        Wk.reshape(6, 128, INNER).transpose(1, 0, 2)).astype(BF16)
    wv_in = np.ascontiguousarray(
        Wv.reshape(6, 128, INNER).transpose(1, 0, 2)).astype(BF16)
    wo_in = np.ascontiguousarray(
        Wo.reshape(4, 128, DQ).transpose(1, 0, 2)).astype(BF16)
    bo_in = bo[None, :].astype(BF16)
    id_in = np.eye(128, dtype=np.float32).astype(BF16)
    ctxT16 = {}
    for bi in range(B):
        ctxT16[bi] = np.ascontiguousarray(
            context[bi].T.reshape(6, 128, S).transpose(1, 0, 2)
        ).astype(BF16)

    in_maps = []
    for core in range(N_CORES):
        xt_core = np.zeros((CAP * TL, DQ), dtype=np.float32)
        m = {}
        off = 0
        for j in range(NSLOT):
            bi, t0 = cores[core][j]
            if bi >= 0:
                r0 = t0 * TL
                r1 = min(r0 + sizes[j] * TL, L)
                if r1 > r0:
                    xt_core[off:off + (r1 - r0)] = x[bi, r0:r1]
                m[f"ctx{j}"] = ctxT16[bi]
            else:
                m[f"ctx{j}"] = ctxT16[0]
            off += sizes[j] * TL
        m["x16"] = np.ascontiguousarray(
            xt_core.reshape(CAP, TL, 2, 128).transpose(3, 2, 0, 1)
        ).astype(BF16)
        m["wq16"] = wq_in
        m["wk16"] = wk_in
        m["wv16"] = wv_in
        m["wo16"] = wo_in
        if has_bias:
            m["bo16"] = bo_in
        m["id16"] = id_in
        in_maps.append(m)

    res = run_bass_kernel_spmd(nc, in_maps, list(range(N_CORES)))

    out = np.zeros((B, L, DQ), dtype=np.float32)
    for core in range(N_CORES):
        yc = res.results[core]["y"]
        off = 0
        for j in range(NSLOT):
            bi, t0 = cores[core][j]
            if bi >= 0:
                r0 = t0 * TL
                r1 = min(r0 + sizes[j] * TL, int(lens[bi]))
                if r1 > r0:
                    out[bi, r0:r1] = yc[off:off + (r1 - r0)]
            off += sizes[j] * TL
    return out


# revision 9
# speedup vs baseline: 1.1784x; 1.0928x over previous
"""CrossAttention Trainium2 kernel (v3: bf16 pipeline, l-major AV).

Reference (per batch b): q = x@Wq; k = ctx@Wk; v = ctx@Wv (H=8, DH=64)
  out = softmax(q k^T / sqrt(DH)) v @ Wo + bo, rows >= seq_len zeroed.

Valid 512-row query tiles are packed across 8 cores with a per-core slot
structure: each core runs CAP tiles; slot j holds sizes[j] consecutive
tiles reading KV buffer j (which batch a (core, slot) holds is data, so
one SPMD program serves all cores). A planner picks slot sizes (up to 3
slots) minimizing CAP then slots; for the staged seq_lens it packs 56
tiles as 8 cores x 7 tiles with zero slot waste.

Engine plan (per tile of 512 queries):
- PE (bf16, 1 cyc/row): qT 4096 + scores 16384 + AV 8320 + transpose
  2048 + oproj 4096 cycles; KV 24576/slot.
- ACT: exp psum->bf16, 16 instrs of [128,2,512] (~16.6us) - bottleneck;
  scores for tile t+1 are emitted before the AV tail of tile t so the
  ACT queue never starves.
- AV is l-major (out [l, h, 65], v augmented with a ones column) so the
  softmax denominator lands as a per-partition scalar: reciprocal
  [128,4,1] + broadcast_to multiply on DVE; then PE-transpose (identity
  matmul) to outT for the bf16 output projection (+ K=1 bias row).
fp8 was measured (numpy mirror of this exact dataflow) at 4e-2..8e-2
rel err - above the 2e-2 gate - so everything stays bf16 (4.9e-3).
"""

import math
import sys

sys.path.insert(0, "/opt/trn_rl_repo")

import numpy as np
import ml_dtypes

B, L, S = 8, 8192, 512
DQ, DC = 256, 768
H, DH = 8, 64
INNER = H * DH
TL = 512
N_CORES = 8

BF16 = ml_dtypes.bfloat16
EXP_SCALE = 1.0 / 8.0          # 1/sqrt(DH)


def _plan(nt):
    """Pick slot sizes (<=3 slots) and per-core pieces.

    Returns (sizes, cores): sizes = tuple of slot lengths (tiles); cores =
    list of 8 entries, each a list of len(sizes) pieces (batch, tile0),
    batch -1 = padding."""
    T = sum(nt)
    lo = max(1, math.ceil(T / N_CORES))
    order = sorted(range(len(nt)), key=lambda i: -nt[i])

    def combos(n, sizes):
        k = len(sizes)
        out = []
        maxx = [min(8, math.ceil(n / s) + 1) for s in sizes]

        def rec(j, x, tot):
            if j == k:
                if tot >= n and all(
                    x[i] == 0 or tot - sizes[i] < n for i in range(k)
                ):
                    out.append(tuple(x))
                return
            for c in range(0, maxx[j] + 1):
                x.append(c)
                rec(j + 1, x, tot + c * sizes[j])
                x.pop()

        rec(0, [], 0)
        return out

    def feasible(sizes):
        k = len(sizes)
        budget = [N_CORES] * k
        pick = [None] * len(nt)

        def rec(bi):
            if bi == len(nt):
                return True
            i = order[bi]
            for x in combos(nt[i], sizes):
                if all(budget[j] >= x[j] for j in range(k)):
                    for j in range(k):
                        budget[j] -= x[j]
                    pick[i] = x
                    if rec(bi + 1):
                        return True
                    for j in range(k):
                        budget[j] += x[j]
            return False

        return pick if rec(0) else None

    best = None
    for cap in range(lo, lo + 10):
        for k in (1, 2, 3):
            parts = set()
            if k == 1:
                parts.add((cap,))
            elif k == 2:
                for a in range(cap - 1, 0, -1):
                    if a >= cap - a:
                        parts.add((a, cap - a))
            else:
                for a in range(cap - 2, 0, -1):
                    for b in range(min(a, cap - a - 1), 0, -1):
                        c = cap - a - b
                        if 0 < c <= b:
                            parts.add((a, b, c))
            for sizes in sorted(parts, reverse=True):
                pick = feasible(sizes)
                if pick is not None:
                    best = (sizes, pick)
                    break
            if best:
                break
        if best:
            break
    assert best is not None
    sizes, pick = best
    k = len(sizes)
    slot_pieces = [[] for _ in range(k)]
    for i, n in enumerate(nt):
        off = 0
        for j in range(k):
            for _ in range(pick[i][j]):
                slot_pieces[j].append((i, off))
                off += sizes[j]
    for j in range(k):
        while len(slot_pieces[j]) < N_CORES:
            slot_pieces[j].append((-1, 0))
    cores = [[slot_pieces[j][c] for j in range(k)] for c in range(N_CORES)]
    return sizes, cores


_PROG_CACHE = {}


def _build_program(sizes, has_bias):
    import concourse.mybir as mybir
    import concourse.tile as tile
    from concourse import bacc

    f32 = mybir.dt.float32
    bf16 = mybir.dt.bfloat16
    Exp = mybir.ActivationFunctionType.Exp
    NSLOT = len(sizes)
    CAP = sum(sizes)
    slot_of = []
    for j, s in enumerate(sizes):
        slot_of += [j] * s

    nc = bacc.Bacc("TRN2", target_bir_lowering=False, debug=False,
                   num_devices=N_CORES)
    x16 = nc.declare_dram_parameter("x16", [128, 2, CAP, TL], bf16,
                                    isOutput=False)
    ctxs = [nc.declare_dram_parameter(f"ctx{j}", [128, 6, S], bf16,
                                      isOutput=False) for j in range(NSLOT)]
    wq16 = nc.declare_dram_parameter("wq16", [128, 2, INNER], bf16,
                                     isOutput=False)
    wk16 = nc.declare_dram_parameter("wk16", [128, 6, INNER], bf16,
                                     isOutput=False)
    wv16 = nc.declare_dram_parameter("wv16", [128, 6, INNER], bf16,
                                     isOutput=False)
    wo16 = nc.declare_dram_parameter("wo16", [128, 4, DQ], bf16,
                                     isOutput=False)
    if has_bias:
        bo16 = nc.declare_dram_parameter("bo16", [1, DQ], bf16,
                                         isOutput=False)
    id16 = nc.declare_dram_parameter("id16", [128, 128], bf16,
                                     isOutput=False)
    y = nc.declare_dram_parameter("y", [CAP * TL, DQ], f32, isOutput=True)

    with tile.TileContext(nc) as tc:
        with (
            tc.tile_pool(name="wpool", bufs=1) as wpool,
            tc.tile_pool(name="kvpool", bufs=1) as kvpool,
            tc.tile_pool(name="mpool", bufs=3) as mpool,
            tc.tile_pool(name="qpool", bufs=2) as qpool,
            tc.tile_pool(name="epool", bufs=2) as epool,
            tc.tile_pool(name="apool", bufs=2) as apool,
            tc.tile_pool(name="opool", bufs=2) as opool,
            tc.tile_pool(name="spool", bufs=2) as spool,
            tc.tile_pool(name="ypool", bufs=3) as ypool,
            tc.tile_pool(name="ps_sc", bufs=2, space="PSUM") as ps_sc,
            tc.tile_pool(name="ps_av", bufs=1, space="PSUM") as ps_av,
            tc.tile_pool(name="ps_q", bufs=1, space="PSUM") as ps_q,
            tc.tile_pool(name="ps_tr", bufs=1, space="PSUM") as ps_tr,
        ):
            # ---- weights / constants (DMA order ~ first-use order) ----
            wk_sb = wpool.tile([128, 6, INNER], bf16, tag="wk", name="wk")
            nc.sync.dma_start(wk_sb[:], wk16[:])
            ctx_sb = [kvpool.tile([128, 6, S], bf16, tag=f"ctx{j}",
                                  name=f"ctx{j}") for j in range(NSLOT)]
            nc.sync.dma_start(ctx_sb[0][:], ctxs[0][:])
            wq_sb = wpool.tile([128, 2, INNER], bf16, tag="wq", name="wq")
            nc.sync.dma_start(wq_sb[:], wq16[:])
            wv_sb = wpool.tile([128, 6, INNER], bf16, tag="wv", name="wv")
            nc.sync.dma_start(wv_sb[:], wv16[:])
            pre_x = {}
            for t in range(min(2, CAP)):
                xt = mpool.tile([128, 2, TL], bf16, tag="x", name="x")
                nc.sync.dma_start(xt[:], x16[:, :, t, :])
                pre_x[t] = xt
            for j in range(1, NSLOT):
                nc.sync.dma_start(ctx_sb[j][:], ctxs[j][:])
            wo_sb = wpool.tile([128, 4, DQ], bf16, tag="wo", name="wo")
            nc.sync.dma_start(wo_sb[:], wo16[:])
            if has_bias:
                bo_sb = wpool.tile([1, DQ], bf16, tag="bo", name="bo")
                nc.sync.dma_start(bo_sb[:], bo16[:])
                ones1 = wpool.tile([1, 128], bf16, tag="on", name="on")
                nc.gpsimd.memset(ones1[:], 1.0)
            id_sb = wpool.tile([128, 128], bf16, tag="id", name="id")
            nc.sync.dma_start(id_sb[:], id16[:])

            # ---- KV pieces per slot (emitted chunked via the work queue) ----
            kT = {}
            vT = {}

            def kv_alloc(j):
                # kT[j]: [128(par*64+d), 4(hpair), S] bf16
                kT[j] = kvpool.tile([128, 4, S], bf16, tag=f"kT{j}",
                                    name=f"kT{j}")
                # vT[j]: [128(s), 4(schunk), 8(h), 65] bf16, col 64 = 1.0
                vT[j] = kvpool.tile([128, 4, 8, DH + 1], bf16, tag=f"v{j}",
                                    name=f"v{j}")
                nc.gpsimd.memset(vT[j][:, :, :, DH:DH + 1], 1.0)

            def kv_kt_chunk(j, m):
                pk = ps_q.tile([128, TL], f32, tag="pq", name="pq")
                for kc in range(6):
                    nc.tensor.matmul(
                        pk[:], wk_sb[:, kc, m * 128:(m + 1) * 128],
                        ctx_sb[j][:, kc, :], start=(kc == 0), stop=(kc == 5))
                nc.vector.tensor_copy(kT[j][:, m, :], pk[:])

            def kv_v_chunk(j, sc):
                pv = ps_q.tile([128, TL], f32, tag="pq", name="pq")
                for kc in range(6):
                    nc.tensor.matmul(
                        pv[:], ctx_sb[j][:, kc, sc * 128:(sc + 1) * 128],
                        wv_sb[:, kc, :], start=(kc == 0), stop=(kc == 5))
                nc.vector.tensor_copy(
                    vT[j][:, sc, :, 0:DH],
                    pv[:].rearrange("p (h d) -> p h d", d=DH))

            # ---- per-tile pieces ----
            def qt_alloc():
                return [qpool.tile([128, TL], bf16, tag=f"q{m}",
                                   name=f"q{m}") for m in range(4)]

            def qt_chunk(qT, xt, m):
                pq = ps_q.tile([128, TL], f32, tag="pq", name="pq")
                for kc in range(2):
                    nc.tensor.matmul(
                        pq[:], wq_sb[:, kc, m * 128:(m + 1) * 128],
                        xt[:, kc, :], start=(kc == 0), stop=(kc == 1))
                nc.vector.tensor_copy(qT[m][:], pq[:])

            def scores_head(j, qT, e, h):
                c, par = h // 2, h % 2
                rhs = qT[c][64 * par:64 * par + 64, :]
                eh = []
                for g in range(2):
                    psc = ps_sc.tile([128, 2, TL], f32, tag="sc", name="sc")
                    for i in range(2):
                        sc = 2 * g + i
                        nc.tensor.matmul(
                            psc[:, i, :],
                            kT[j][64 * par:64 * par + 64, c,
                                  sc * 128:(sc + 1) * 128],
                            rhs, start=True, stop=True)
                    e16 = epool.tile([128, 2, TL], bf16, tag=f"e{h}{g}",
                                     name=f"e{h}{g}")
                    nc.scalar.activation(e16[:], psc[:], Exp,
                                         scale=EXP_SCALE)
                    eh.append(e16)
                e.append(eh)

            # tail chunks for tile t (13 closures)
            def tail_chunks(t, j, e):
                st = {}

                def av_c(lc):
                    pav = [ps_av.tile([128, 4, DH + 1], f32, tag=f"va{a}",
                                      name=f"va{a}") for a in range(2)]
                    st[lc] = pav
                    for h in range(H):
                        a, hh = h // 4, h % 4
                        for sc in range(4):
                            nc.tensor.matmul(
                                pav[a][:, hh, :],
                                e[h][sc // 2][:, sc % 2,
                                              lc * 128:(lc + 1) * 128],
                                vT[j][:, sc, h, :],
                                start=(sc == 0), stop=(sc == 3))

                def norm_c(lc):
                    pav = st.pop(lc)
                    rp = spool.tile([128, 8, 1], f32, tag="rp", name="rp")
                    av = apool.tile([128, 8, DH], bf16, tag=f"av{lc}",
                                    name=f"av{lc}")
                    st[("av", lc)] = av
                    for a in range(2):
                        nc.vector.reciprocal(rp[:, 4 * a:4 * a + 4, :],
                                             pav[a][:, :, DH:DH + 1])
                        nc.vector.tensor_tensor(
                            av[:, 4 * a:4 * a + 4, :], pav[a][:, :, 0:DH],
                            rp[:, 4 * a:4 * a + 4, :].broadcast_to(
                                [128, 4, DH]),
                            mybir.AluOpType.mult)

                def tr_c(lc):
                    av = st.pop(("av", lc))
                    ptr = ps_tr.tile([128, 4, 128], bf16, tag="tr",
                                     name="tr")
                    for ic in range(4):
                        nc.tensor.transpose(
                            ptr[:, ic, :], av[:, 2 * ic:2 * ic + 2, :],
                            id_sb[:])
                    nc.vector.tensor_copy(st["outT"][:, :, lc, :], ptr[:])

                def oproj_c(ls):
                    outT = st["outT"]
                    po = ps_q.tile([128, TL], f32, tag="pq", name="pq")
                    for kc in range(4):
                        nc.tensor.matmul(po[:, 0:DQ], outT[:, kc, ls, :],
                                         wo_sb[:, kc, :], start=(kc == 0),
                                         stop=(kc == 3 and not has_bias))
                    if has_bias:
                        nc.tensor.matmul(po[:, 0:DQ], ones1[:], bo_sb[:],
                                         start=False, stop=True)
                    yt = ypool.tile([128, DQ], f32, tag="y", name="y")
                    nc.vector.tensor_copy(yt[:], po[:, 0:DQ])
                    nc.sync.dma_start(
                        y[t * TL + ls * 128:t * TL + (ls + 1) * 128, :],
                        yt[:])

                def alloc_outT():
                    st["outT"] = opool.tile([128, 4, 4, 128], bf16,
                                            tag="oT", name="oT")

                return ([lambda lc=0: av_c(0)]
                        + [lambda lc=lc: (norm_c(lc - 1), av_c(lc))
                           for lc in range(1, 4)]
                        + [lambda: (norm_c(3), alloc_outT())]
                        + [lambda lc=lc: tr_c(lc) for lc in range(4)]
                        + [lambda ls=ls: oproj_c(ls) for ls in range(4)])

            # ---- main loop: deadline-ordered work weave ----
            import heapq
            work = []       # heap of (deadline, seq, closure)
            seqn = [0]

            def push(dl, fn):
                heapq.heappush(work, (dl, seqn[0], fn))
                seqn[0] += 1

            def pump(n):
                for _ in range(n):
                    if not work:
                        return
                    heapq.heappop(work)[2]()

            def drain(i):
                while work and work[0][0] <= i:
                    heapq.heappop(work)[2]()

            first_tile = {}
            for t in range(CAP):
                first_tile.setdefault(slot_of[t], t)

            # startup: kv0 kT inline (scores(0) needs it); vT(0) queued
            kv_alloc(0)
            for m in range(4):
                kv_kt_chunk(0, m)
            for sc in range(4):
                push(1, lambda j=0, sc=sc: kv_v_chunk(j, sc))
            # enqueue later slots' kv early (deadline = their first tile)
            for j in range(1, NSLOT):
                kv_alloc(j)
                ft = first_tile[j]
                for m in range(4):
                    push(ft, lambda j=j, m=m: kv_kt_chunk(j, m))
                for sc in range(4):
                    push(ft, lambda j=j, sc=sc: kv_v_chunk(j, sc))

            qT_cur = qt_alloc()
            for m in range(4):
                qt_chunk(qT_cur, pre_x[0], m)

            for t in range(CAP):
                j = slot_of[t]
                drain(t)
                if t + 2 < CAP:
                    xt2 = mpool.tile([128, 2, TL], bf16, tag="x", name="x")
                    nc.sync.dma_start(xt2[:], x16[:, :, t + 2, :])
                    pre_x[t + 2] = xt2
                if t + 1 < CAP:
                    qT_nxt = qt_alloc()
                    xt_n = pre_x[t + 1]
                    for m in range(4):
                        push(t + 1,
                             lambda q=qT_nxt, x=xt_n, m=m: qt_chunk(q, x, m))
                e = []
                for h in range(H):
                    scores_head(j, qT_cur, e, h)
                    if h >= 1:
                        n = 2 if len(work) <= 2 * (H - h) else 3
                        pump(n)
                for c in tail_chunks(t, j, e):
                    push(t + 2, c)
                if t + 1 < CAP:
                    qT_cur = qT_nxt
            drain(CAP + 2)
    nc.compile()
    return nc


def kernel(x, context, seq_lens, Wq, Wk, Wv, Wo, bo):
    from concourse.bass_utils import run_bass_kernel_spmd

    x = np.asarray(x, dtype=np.float32)
    context = np.asarray(context, dtype=np.float32)
    seq_lens = np.asarray(seq_lens, dtype=np.int32)
    Wq = np.asarray(Wq, dtype=np.float32)
    Wk = np.asarray(Wk, dtype=np.float32)
    Wv = np.asarray(Wv, dtype=np.float32)
    Wo = np.asarray(Wo, dtype=np.float32)
    bo = np.asarray(bo, dtype=np.float32)

    lens = np.clip(seq_lens, 1, L)
    nt = [int(math.ceil(int(n) / TL)) for n in lens]
    sizes, cores = _plan(nt)
    NSLOT = len(sizes)
    CAP = sum(sizes)
    has_bias = bool(np.any(bo != 0.0))

    key = (sizes, has_bias)
    if key not in _PROG_CACHE:
        _PROG_CACHE[key] = _build_program(sizes, has_bias)
    nc = _PROG_CACHE[key]

    wq_in = np.ascontiguousarray(
        Wq.reshape(2, 128, INNER).transpose(1, 0, 2)).astype(BF16)
    wk_in = np.ascontiguousarray(
        Wk.reshape(6, 128, INNER).transpose(1, 0, 2)).astype(BF16)
    wv_in = np.ascontiguousarray(
        Wv.reshape(6, 128, INNER).transpose(1, 0, 2)).astype(BF16)
    wo_in = np.ascontiguousarray(
        Wo.reshape(4, 128, DQ).transpose(1, 0, 2)).astype(BF16)
    bo_in = bo[None, :].astype(BF16)
    id_in = np.eye(128, dtype=np.float32).astype(BF16)
    ctxT16 = {}
    for bi in range(B):
        ctxT16[bi] = np.ascontiguousarray(
            context[bi].T.reshape(6, 128, S).transpose(1, 0, 2)
        ).astype(BF16)

    in_maps = []
    for core in range(N_CORES):
        xt_core = np.zeros((CAP * TL, DQ), dtype=np.float32)
        m = {}
        off = 0
        for j in range(NSLOT):
            bi, t0 = cores[core][j]
            if bi >= 0:
                r0 = t0 * TL
                r1 = min(r0 + sizes[j] * TL, L)
                if r1 > r0:
                    xt_core[off:off + (r1 - r0)] = x[bi, r0:r1]
                m[f"ctx{j}"] = ctxT16[bi]
            else:
                m[f"ctx{j}"] = ctxT16[0]
            off += sizes[j] * TL
        m["x16"] = np.ascontiguousarray(
            xt_core.reshape(CAP, TL, 2, 128).transpose(3, 2, 0, 1)
        ).astype(BF16)
        m["wq16"] = wq_in
        m["wk16"] = wk_in
        m["wv16"] = wv_in
        m["wo16"] = wo_in
        if has_bias:
            m["bo16"] = bo_in
        m["id16"] = id_in
        in_maps.append(m)

    res = run_bass_kernel_spmd(nc, in_maps, list(range(N_CORES)))

    out = np.zeros((B, L, DQ), dtype=np.float32)
    for core in range(N_CORES):
        yc = res.results[core]["y"]
        off = 0
        for j in range(NSLOT):
            bi, t0 = cores[core][j]
            if bi >= 0:
                r0 = t0 * TL
                r1 = min(r0 + sizes[j] * TL, int(lens[bi]))
                if r1 > r0:
                    out[bi, r0:r1] = yc[off:off + (r1 - r0)]
            off += sizes[j] * TL
    return out
